# revision 42
# baseline (speedup 1.0000x reference)
"""GAT (2-layer PyG GATConv, eval) on 8 Trainium2 NeuronCores.

Sharding: nodes range-partitioned (NLOC=12800/core); core c owns edges whose
dst is in its range. Both layers' node tables are computed SHARDED (each core
transforms only its own 12800-node block) and replicated by one AllGather per
layer; each AllGather is fully overlapped by the a_dst gather burst for the
next edge pass, which reads only the local block.

Slot layout per core: superblock (10 windows) -> quadrant -> window, with
per-(window,quadrant) STATIC capacities = max edge count over the 8 cores
(SPMD: one module runs on all cores; only tensor contents differ) — ~10%
slot padding vs 28% for fixed-size groups. Every dma_gather call carries at
most 1024 indices (the SWDGE ucode's ring limit on real hardware; larger
calls hard-crash the device), rotated across the 4 SWDGE queues.

Per layer, per edge slot: a 144B/82B payload gather pulls [h|a_src] rows
(256B-stride tables, int16 idx into 25600-row quadrants); a 16B/4B gather
pulls a_dst. Softmax is the shift-invariant no-max form
(w = exp(leakyrelu(as+ad)), |e| < ~25 so fp32 exp is safe). Segment reduction
is a PE matmul whose stationary matrix is a transposed one-hot built by DVE
is_equal in the 2x-mode layout (both operands packed 2-byte, last-dim stride
1); chunk slices address it with a strided free dim, boundary chunks carry a
masked drel column per touching window. Messages msg = h*w use an
Act-expanded weight tile so the DVE mult also runs in 2x mode. Softmax
normalize + ELU + the r2 = hlT @ [W2|a2_src|a2_dst] projection are batched
per superblock; tables are written node-permuted (row = p*100 + j within
each core block) so table writes coalesce to one descriptor per partition.
"""
import numpy as np
import ml_dtypes

N = 100000
E = 1600000
NF = 256
HEADS, NHID = 8, 8
NH = HEADS * NHID          # 64
NCLASS = 40
NLOC = 12800               # nodes per core
NW = 100                   # 128-dst windows per core
NQ = 4                     # src table quadrants
QS = 25600                 # rows per quadrant
NWSB = 10                  # windows per superblock
NSB = NW // NWSB           # 10 superblocks
NTOT = 102400
ACC_EPS = 1e-16

_CACHE = {}


def _ceil128(x):
    return (x + 127) & ~127


def _host_prep(x, edge_index, W1, a1_src, a1_dst, b1, W2, a2_src, a2_dst, b2):
    src = np.asarray(edge_index[0], dtype=np.int64)
    dst = np.asarray(edge_index[1], dtype=np.int64)

    # table-row permutation: node n -> row  c*NLOC + (l%128)*NW + l//128
    def rowperm(n):
        c = n // NLOC
        l = n - c * NLOC
        return c * NLOC + (l % 128) * NW + l // 128

    srow = rowperm(src)
    sq = srow // QS
    sidx = (srow - sq * QS).astype(np.int16)

    core = dst // NLOC
    dloc = dst - core * NLOC
    w_e = (dloc >> 7).astype(np.int64)
    dr_e = (dloc & 127).astype(np.int64)
    adidx = (dr_e * NW + w_e).astype(np.int16)
    sb_e = w_e // NWSB

    # static capacities: max over cores per (window, quadrant)
    gkey = (core * NW + w_e) * NQ + sq          # [E]
    cnt = np.bincount(gkey, minlength=8 * NW * NQ).reshape(8, NW, NQ)
    cap = cnt.max(axis=0)                        # [NW, NQ]

    # slot layout: sb -> quadrant -> window.  SWDGE gather ucode dies above
    # 1024 indices per call (empirical), so every call is <= 1024.
    MAXIDX = 1024
    wq_start = np.zeros((NW, NQ), np.int64)
    hcalls = []                                  # (sb, q, slot0, n_idx)
    sb_chunks = []                               # (k0, k1) per sb
    nslot = 0
    for s in range(NSB):
        k0 = nslot // 128
        for q in range(NQ):
            seg0 = nslot
            for w in range(s * NWSB, (s + 1) * NWSB):
                wq_start[w, q] = nslot
                nslot += int(cap[w, q])
            nslot = _ceil128(nslot)
            for off in range(seg0, nslot, MAXIDX):
                hcalls.append((s, q, off, min(MAXIDX, nslot - off)))
        sb_chunks.append((k0, nslot // 128))
    NSLOT = nslot
    NCHUNK = NSLOT // 128

    # dst-indexed gather calls (a_dst expansion), whole slot range
    adcalls = [(off, min(MAXIDX, NSLOT - off))
               for off in range(0, NSLOT, MAXIDX)]

    # per-window chunk columns
    colmap = np.full((NW, NCHUNK), -1, np.int64)
    wcols = []                                   # per w: (colbase, [chunks])
    ncol = 0
    for w in range(NW):
        cols = []
        for q in range(NQ):
            a = int(wq_start[w, q])
            b = a + int(cap[w, q])
            for k in range(a // 128, (b + 127) // 128):
                cols.append(k)
                colmap[w, k] = ncol
                ncol += 1
        wcols.append(cols)
    NCOL = ncol
    MAXCPW = max(len(c) for c in wcols)

    plan = {
        "NSLOT": NSLOT, "NCHUNK": NCHUNK, "NCOL": NCOL, "MAXCPW": MAXCPW,
        "hcalls": hcalls, "adcalls": adcalls, "sb_chunks": sb_chunks,
        "wcols": wcols,
        "skip_b1": bool(np.all(np.asarray(b1) == 0)),
        "skip_b2": bool(np.all(np.asarray(b2) == 0)),
    }

    # group-id in slot order: (sb, q, w_in_sb)
    flatg = (sb_e * NQ + sq) * NWSB + (w_e - sb_e * NWSB)
    gstart_flat = np.zeros(NSB * NQ * NWSB, np.int64)
    for s in range(NSB):
        for q in range(NQ):
            for wi in range(NWSB):
                gstart_flat[(s * NQ + q) * NWSB + wi] = wq_start[s * NWSB + wi, q]

    per_core = []
    hidx_all, adidx_all, drel_all = [], [], []
    for c in range(8):
        m = core == c
        fg = flatg[m]
        drc = dr_e[m]
        order = np.lexsort((drc, fg))
        fgs = fg[order]
        cntc = np.bincount(fgs, minlength=NSB * NQ * NWSB)
        starts = np.zeros_like(cntc)
        starts[1:] = np.cumsum(cntc)[:-1]
        rank = np.arange(len(fgs)) - starts[fgs]
        slot = gstart_flat[fgs] + rank

        hvec = np.zeros(NSLOT, np.int16)
        avec = np.zeros(NSLOT, np.int16)
        hvec[slot] = sidx[m][order]
        avec[slot] = adidx[m][order]

        drel = np.full((128, NCOL), 128.0, np.float32)
        k_s = slot >> 7
        p_s = slot & 127
        we_s = w_e[m][order]
        col_s = colmap[we_s, k_s]
        assert (col_s >= 0).all()
        drel[p_s, col_s] = drc[order].astype(np.float32)

        def wrap16(v):
            o = np.zeros((128, NSLOT // 16), np.int16)
            sl = np.arange(NSLOT)
            o[sl % 16, sl // 16] = v
            for r in range(1, 8):
                o[16 * r:16 * (r + 1)] = o[:16]
            return o

        hidx_all.append(wrap16(hvec))
        adidx_all.append(wrap16(avec))
        drel_all.append(drel.astype(ml_dtypes.bfloat16))

    # weights
    W1 = np.asarray(W1, np.float32)
    v_s1 = np.einsum("chk,hk->ch", W1.reshape(NF, HEADS, NHID),
                     np.asarray(a1_src, np.float32))
    v_d1 = np.einsum("chk,hk->ch", W1.reshape(NF, HEADS, NHID),
                     np.asarray(a1_dst, np.float32))
    W1e = np.concatenate([W1, v_s1, v_d1], axis=1).reshape(2, 128, 80)
    W1e = W1e.astype(np.float16)

    W2 = np.asarray(W2, np.float32)
    v_s2 = W2 @ np.asarray(a2_src, np.float32)[0]
    v_d2 = W2 @ np.asarray(a2_dst, np.float32)[0]
    W2e = np.concatenate([W2, v_s2[:, None], v_d2[:, None]],
                         axis=1).astype(np.float16)   # [64, 42]

    xp = np.zeros((NTOT, NF), np.float32)
    xp[:N] = np.asarray(x, np.float32)

    for c in range(8):
        xloc = np.ascontiguousarray(xp[c * NLOC:(c + 1) * NLOC].T)
        per_core.append({
            "xTloc": xloc.astype(np.float16).reshape(2, 128, NLOC),
            "W1e": W1e,
            "W2e": W2e,
            "b1": np.asarray(b1, np.float32)[None, :],
            "b2": np.asarray(b2, np.float32)[None, :],
            "hidx": hidx_all[c],
            "adidx": adidx_all[c],
            "drel": drel_all[c],
        })
    return per_core, plan


def _gather_small(g, out_ap, in_ap, idxs_ap, num_idxs, elem_size, elem_step,
                  queue_num=0):
    """dma_gather with payload < 256B; only the 256B row-stride rule is real
    for the non-transpose path."""
    import concourse.mybir as mybir
    stride_bytes = elem_step * mybir.dt.size(in_ap.dtype)
    assert stride_bytes % 256 == 0
    _in_ap = g.lower_ap_dma(in_ap, for_custom_bir_dma=True)
    _idxs_ap = g.lower_ap(idxs_ap)
    _out_ap = g.lower_ap(out_ap)
    return g.add_instruction(mybir.InstDMAGatherAnt(
        name=g.bass.get_next_instruction_name(),
        ins=[*_in_ap, _idxs_ap, g.lower_val_access(g.to_reg(num_idxs))],
        outs=[_out_ap],
        transpose=False,
        num_idxs=num_idxs,
        elem_size=elem_size,
        stride_bytes_256=stride_bytes // 256,
        gen_mode=0,
        single_packet=True,
        queue_num=queue_num,
        sbuf_tokens_per_rank=0,
        sbuf_free_dim_per_rank=0,
        sbuf_free_dim_pad_per_rank=0,
        sbuf_byte_offset=0,
    ))


def _build_nc(plan):
    import concourse.bass as bass
    import concourse.bacc as bacc
    import concourse.mybir as mybir
    import concourse.tile as tile
    from concourse.library_config import mlp
    from concourse.masks import make_identity

    f32, f16, bf16, i16 = (mybir.dt.float32, mybir.dt.float16,
                           mybir.dt.bfloat16, mybir.dt.int16)
    AF = mybir.ActivationFunctionType
    OP = mybir.AluOpType

    NSLOT = plan["NSLOT"]
    NCOL = plan["NCOL"]
    MAXCPW = plan["MAXCPW"]
    sb_chunks = plan["sb_chunks"]
    wcols = plan["wcols"]
    CPSB_MAX = max(k1 - k0 for k0, k1 in sb_chunks)
    colbase = [0] * NW
    for w in range(1, NW):
        colbase[w] = colbase[w - 1] + len(wcols[w - 1])

    nc = bacc.Bacc("TRN2", target_bir_lowering=False, debug=False,
                   num_devices=8, num_swdge_queues=4)

    xTloc = nc.dram_tensor("xTloc", [2, 128, NLOC], f16, kind="ExternalInput")
    W1e = nc.dram_tensor("W1e", [2, 128, 80], f16, kind="ExternalInput")
    W2e = nc.dram_tensor("W2e", [64, 42], f16, kind="ExternalInput")
    b1 = nc.dram_tensor("b1", [1, 64], f32, kind="ExternalInput")
    b2 = nc.dram_tensor("b2", [1, 40], f32, kind="ExternalInput")
    hidx_d = nc.dram_tensor("hidx", [128, NSLOT // 16], i16,
                            kind="ExternalInput")
    adidx_d = nc.dram_tensor("adidx", [128, NSLOT // 16], i16,
                             kind="ExternalInput")
    drel_d = nc.dram_tensor("drel", [128, NCOL], bf16, kind="ExternalInput")
    out = nc.dram_tensor("out", [NLOC, 40], f32, kind="ExternalOutput")

    agi1 = nc.dram_tensor("agi1", [NLOC, 128], f16)    # local [h1|as1|ad1|pad]
    tab1 = nc.dram_tensor("tab1", [NTOT, 128], f16, addr_space="Shared")
    agi2 = nc.dram_tensor("agi2", [NLOC, 128], f16)    # [h2|as2|ad2|pad]
    ago = nc.dram_tensor("ago", [NTOT, 128], f16, addr_space="Shared")

    def BC(ap, dims):
        return bass.AP(ap.tensor, ap.offset, dims)

    def dram_rows(t, offset_rows, dims):
        """AP into DRAM tensor t (row-major, 128 f16 cols) at row offset."""
        return bass.AP(t, offset_rows * 128, dims)

    with tile.TileContext(nc) as tc:
        with tc.tile_pool(name="const", bufs=1) as pc:
            nc.gpsimd.load_library(mlp)

            drel_sb = pc.tile([128, NCOL], bf16)
            nc.sync.dma_start(drel_sb[:], drel_d[:])
            w1_sb = pc.tile([128, 2, 80], f16)
            nc.sync.dma_start(w1_sb[:], W1e[:].rearrange("k p n -> p k n"))
            w2_sb = pc.tile([64, 42], f16)
            nc.sync.dma_start(w2_sb[:], W2e[:])

            ii = pc.tile([128, 128, MAXCPW], i16)
            nc.gpsimd.iota(ii[:], pattern=[[1, 128], [0, MAXCPW]], base=0,
                           channel_multiplier=0)
            iota_rep = pc.tile([128, 128, MAXCPW], bf16)
            nc.vector.tensor_copy(out=iota_rep[:], in_=ii[:])

            ident = pc.tile([128, 128], f16)
            make_identity(nc, ident[:])

            ones32 = pc.tile([1, 128], f32)
            nc.vector.memset(ones32[:], 1.0)

            b1r = pc.tile([128, 64], f32)
            b2r = pc.tile([128, 40], f32)
            with tc.tile_pool(name="pini", bufs=2, space="PSUM") as ppi:
                for row_d, width, dest in ((b1, 64, b1r), (b2, 40, b2r)):
                    t = pc.tile([1, width], f32, tag=f"rrow{width}")
                    nc.sync.dma_start(t[:], row_d[:])
                    ps = ppi.tile([128, width], f32, tag="rep")
                    nc.tensor.matmul(ps[:], lhsT=ones32[:], rhs=t[:],
                                     start=True, stop=True)
                    nc.vector.tensor_copy(out=dest[:], in_=ps[:])

            # a_dst per-slot tiles, gathered once and reused per layer
            at1_all = pc.tile([128, plan["NCHUNK"], 8], f16)
            at2_all = pc.tile([128, plan["NCHUNK"], 2], f16)

            # ---------- phase A (sharded): each core transforms only its own
            # node block -> agi1, AllGather -> tab1; the ad1-gather burst runs
            # during the collective (it reads the local agi1) ----------
            with (tc.tile_pool(name="pa", bufs=3) as pa,
                  tc.tile_pool(name="ppa", bufs=2, space="PSUM") as ppa):
                AB = 16                      # chunks per DMA batch
                for jj in range(0, 100, AB):
                    nb = min(AB, 100 - jj)
                    xt = pa.tile([128, 2, AB * 128], f16, tag="xt")
                    for k in range(2):
                        nc.sync.dma_start(
                            xt[:, k, 0:nb * 128],
                            xTloc[k, :, jj * 128:(jj + nb) * 128])
                    row = pa.tile([128, AB, 128], f16, tag="row")
                    for u in range(0, nb, 4):
                        ub = min(4, nb - u)
                        ps = ppa.tile([128, 4, 80], f32, tag="np1")
                        for j in range(ub):
                            for k in range(2):
                                nc.tensor.matmul(
                                    ps[:, j, :],
                                    lhsT=xt[:, k,
                                            (u + j) * 128:(u + j + 1) * 128],
                                    rhs=w1_sb[:, k, :], start=(k == 0),
                                    stop=(k == 1))
                        if (u // 4) % 2:
                            nc.vector.tensor_copy(out=row[:, u:u + ub, 0:80],
                                                  in_=ps[:, 0:ub, :])
                        else:
                            nc.scalar.copy(out=row[:, u:u + ub, 0:80],
                                           in_=ps[:, 0:ub, :])
                    nc.sync.dma_start(
                        dram_rows(agi1, jj,
                                  [[NW * 128, 128], [128, nb], [1, 128]]),
                        row[:, 0:nb, :])

                nc.gpsimd.collective_compute(
                    "AllGather", OP.bypass, ins=[agi1[:]], outs=[tab1[:]],
                    replica_groups=[list(range(8))])

                # ad1-gather burst (reads local agi1; overlaps the AllGather)
                aix = pa.tile([128, NSLOT // 16], i16, tag="aix")
                nc.sync.dma_start(aix[:], adidx_d[:])
                for ci, (slot0, nids) in enumerate(plan["adcalls"]):
                    _gather_small(
                        nc.gpsimd,
                        at1_all[:, slot0 // 128:(slot0 + nids) // 128, :],
                        agi1[:, 72:80],
                        aix[:, slot0 // 16:(slot0 + nids) // 16],
                        nids, 8, 128, queue_num=ci % 4)

            # ---------- phase B: layer-1 edge pass ----------
            def edge_pass(layer):
                if layer == 1:
                    tab, ncols_h, as_col = tab1, 72, 64
                    nheads, msgw = 8, 72
                else:
                    tab, ncols_h, as_col = ago, 41, 40
                    nheads, msgw = 1, 41
                pool_name = f"pe{layer}"
                with (tc.tile_pool(name=pool_name, bufs=2) as pb,
                      tc.tile_pool(name=pool_name + "h", bufs=3) as ph,
                      tc.tile_pool(name=pool_name + "m", bufs=1) as pm,
                      tc.tile_pool(name=pool_name + "w",
                                   bufs=(2 if layer == 1 else 3)) as pw,
                      tc.tile_pool(name=pool_name + "p", bufs=2,
                                   space="PSUM") as ppb):
                    qn = 0
                    for s in range(NSB):
                        k0, k1 = sb_chunks[s]
                        cps = k1 - k0
                        hix = pb.tile([128, CPSB_MAX * 8], i16, tag="hix")
                        nc.sync.dma_start(hix[:, 0:cps * 8],
                                          hidx_d[:, k0 * 8:k1 * 8])

                        ht = ph.tile([128, CPSB_MAX, ncols_h], f16, tag="ht")
                        for (ss, q, slot0, nids) in plan["hcalls"]:
                            if ss != s:
                                continue
                            c0 = slot0 // 128 - k0
                            _gather_small(
                                nc.gpsimd,
                                ht[:, c0:c0 + nids // 128, :],
                                tab[q * QS:(q + 1) * QS, 0:ncols_h],
                                hix[:, (slot0 - k0 * 128) // 16:
                                    (slot0 - k0 * 128 + nids) // 16],
                                nids, ncols_h, 128, queue_num=qn % 4)
                            qn += 1
                        if layer == 1:
                            at_s = at1_all[:, k0:k1, :]
                        else:
                            at_s = at2_all[:, k0:k1, 1:2]

                        e = pm.tile([128, CPSB_MAX, nheads], f32, tag="e")
                        lr = e
                        nc.vector.tensor_tensor(
                            out=e[:, 0:cps, :],
                            in0=ht[:, 0:cps, as_col:as_col + nheads],
                            in1=at_s, op=OP.add)
                        nc.vector.scalar_tensor_tensor(
                            out=lr[:, 0:cps, :], in0=e[:, 0:cps, :],
                            scalar=0.2, in1=e[:, 0:cps, :],
                            op0=OP.mult, op1=OP.max)

                        msg = pm.tile([128, CPSB_MAX, msgw], bf16, tag="msg")
                        # w into msg's trailing cols (compact exp)
                        nc.scalar.activation(
                            out=msg[:, 0:cps, as_col:as_col + nheads],
                            in_=lr[:, 0:cps, :], func=AF.Exp)
                        if layer == 1:
                            # expanded weights for a clean 2x-mode mult
                            half = (CPSB_MAX + 1) // 2
                            wgx = pm.tile([128, half, 8, 8], bf16, tag="wgx")
                            for h0 in (0, half):
                                hn = min(half, cps - h0)
                                if hn <= 0:
                                    continue
                                lrs = lr[:, h0:h0 + hn, :]
                                nc.scalar.activation(
                                    out=wgx[:, 0:hn, :, :],
                                    in_=BC(lrs, [lrs.ap[0], lrs.ap[1],
                                                 lrs.ap[2], [0, 8]]),
                                    func=AF.Exp)
                                m_ = msg[:, h0:h0 + hn, 0:64]
                                h_ = ht[:, h0:h0 + hn, 0:64]
                                nc.vector.tensor_tensor(
                                    out=BC(m_, [m_.ap[0], m_.ap[1],
                                                [8, 8], [1, 8]]),
                                    in0=BC(h_, [h_.ap[0], h_.ap[1],
                                                [8, 8], [1, 8]]),
                                    in1=wgx[:, 0:hn, :, :], op=OP.mult)
                        else:
                            wgx2 = pw.tile([128, CPSB_MAX, 40], bf16,
                                           tag="wgx2")
                            lrs = lr[:, 0:cps, :]
                            nc.scalar.activation(
                                out=wgx2[:, 0:cps, :],
                                in_=BC(lrs, [lrs.ap[0], lrs.ap[1], [0, 40]]),
                                func=AF.Exp)
                            nc.vector.tensor_tensor(
                                out=msg[:, 0:cps, 0:40],
                                in0=ht[:, 0:cps, 0:40],
                                in1=wgx2[:, 0:cps, :], op=OP.mult)

                        # windows: one-hot + aggregation matmuls, PSUM
                        # evicted into a per-sb batch tile
                        hsb = pm.tile([128, NWSB, msgw], f32, tag="hsb")
                        for wi in range(NWSB):
                            w = s * NWSB + wi
                            cols = wcols[w]
                            cpw = len(cols)
                            c0 = colbase[w]
                            ohT = pw.tile([128, 128, MAXCPW], bf16, tag="ohT")
                            dr = drel_sb[:, c0:c0 + cpw]
                            nc.vector.tensor_tensor(
                                out=ohT[:, :, 0:cpw],
                                in0=BC(dr, [dr.ap[0], [0, 128], dr.ap[1]]),
                                in1=iota_rep[:, :, 0:cpw], op=OP.is_equal)
                            ps = ppb.tile([128, msgw], f32, tag="agg")
                            for i, k in enumerate(cols):
                                nc.tensor.matmul(
                                    ps[:], lhsT=ohT[:, :, i],
                                    rhs=msg[:, k - k0, :],
                                    start=(i == 0), stop=(i == cpw - 1))
                            nc.scalar.copy(out=hsb[:, wi, :], in_=ps[:])

                        # per-sb batched softmax-normalize (+ elu/r2 for L1)
                        if layer == 1:
                            den = pw.tile([128, NWSB, 8], f32, tag="den")
                            nc.scalar.activation(out=den[:],
                                                 in_=hsb[:, :, 64:72],
                                                 func=AF.Copy, bias=ACC_EPS)
                            rec = pw.tile([128, NWSB, 8], f32, tag="rec")
                            nc.vector.reciprocal(
                                rec[:].rearrange("p a b -> p (a b)"),
                                den[:].rearrange("p a b -> p (a b)"))
                            o1 = pw.tile([128, NWSB, 64], f32, tag="o1")
                            nu = hsb[:, :, 0:64]
                            r_ = rec[:]
                            nc.vector.tensor_tensor(
                                out=BC(o1[:], [o1[:].ap[0], [64, NWSB],
                                               [8, 8], [1, 8]]),
                                in0=BC(nu, [nu.ap[0], [72, NWSB],
                                            [8, 8], [1, 8]]),
                                in1=BC(r_, [r_.ap[0], [8, NWSB],
                                            [1, 8], [0, 8]]),
                                op=OP.mult)
                            o1v = o1[:].rearrange("p a b -> p (a b)")
                            if not plan["skip_b1"]:
                                b1w = b1r[:]
                                nc.vector.tensor_tensor(
                                    out=o1v,
                                    in0=o1v,
                                    in1=BC(b1w, [b1w.ap[0], [0, NWSB],
                                                 [1, 64]]),
                                    op=OP.add)
                            # elu = relu(x) + exp(-relu(-x)) - 1
                            rneg = pw.tile([128, NWSB, 64], f32, tag="rneg")
                            nc.scalar.activation(
                                out=rneg[:].rearrange("p a b -> p (a b)"),
                                in_=o1v, func=AF.Relu, scale=-1.0)
                            expn = rneg
                            nc.scalar.activation(
                                out=expn[:].rearrange("p a b -> p (a b)"),
                                in_=rneg[:].rearrange("p a b -> p (a b)"),
                                func=AF.Exp, scale=-1.0)
                            pos = pw.tile([128, NWSB, 64], f32, tag="pos")
                            nc.scalar.activation(
                                out=pos[:].rearrange("p a b -> p (a b)"),
                                in_=o1v, func=AF.Relu)
                            hl16 = pw.tile([128, NWSB, 64], f16, tag="hl16")
                            nc.vector.scalar_tensor_tensor(
                                out=hl16[:].rearrange("p a b -> p (a b)"),
                                in0=expn[:].rearrange("p a b -> p (a b)"),
                                scalar=-1.0,
                                in1=pos[:].rearrange("p a b -> p (a b)"),
                                op0=OP.add, op1=OP.add)
                            r2s = pw.tile([128, NWSB, 42], f16, tag="r2s")
                            for wi in range(NWSB):
                                pst = ppb.tile([64, 128], f16, tag="tr")
                                nc.tensor.transpose(out=pst[:],
                                                    in_=hl16[:, wi, :],
                                                    identity=ident[:])
                                hlT = pw.tile([64, 128], f16, tag="hlT")
                                nc.scalar.copy(out=hlT[:], in_=pst[:])
                                r2p = ppb.tile([128, 42], f32, tag="r2p")
                                nc.tensor.matmul(r2p[:], lhsT=hlT[:],
                                                 rhs=w2_sb[:], start=True,
                                                 stop=True)
                                nc.scalar.copy(out=r2s[:, wi, :], in_=r2p[:])
                            nc.sync.dma_start(
                                bass.AP(agi2, (s * NWSB) * 128,
                                        [[NW * 128, 128], [128, NWSB],
                                         [1, 42]]),
                                r2s[:])
                        else:
                            den = pw.tile([128, NWSB, 1], f32, tag="den2")
                            nc.scalar.activation(out=den[:],
                                                 in_=hsb[:, :, 40:41],
                                                 func=AF.Copy, bias=ACC_EPS)
                            rec = pw.tile([128, NWSB, 1], f32, tag="rec2")
                            nc.vector.reciprocal(
                                rec[:].rearrange("p a b -> p (a b)"),
                                den[:].rearrange("p a b -> p (a b)"))
                            o2 = pw.tile([128, NWSB, 40], f32, tag="o2")
                            nu = hsb[:, :, 0:40]
                            r_ = rec[:]
                            nc.vector.tensor_tensor(
                                out=o2[:],
                                in0=BC(nu, [nu.ap[0], [41, NWSB], [1, 40]]),
                                in1=BC(r_, [r_.ap[0], [1, NWSB], [0, 40]]),
                                op=OP.mult)
                            o2v = o2[:].rearrange("p a b -> p (a b)")
                            if not plan["skip_b2"]:
                                b2w = b2r[:]
                                nc.vector.tensor_tensor(
                                    out=o2v, in0=o2v,
                                    in1=BC(b2w, [b2w.ap[0], [0, NWSB],
                                                 [1, 40]]),
                                    op=OP.add)
                            nc.sync.dma_start(
                                bass.AP(out, (s * NWSB) * 128 * 40,
                                        [[40, 128], [128 * 40, NWSB],
                                         [1, 40]]),
                                o2[:])

            edge_pass(1)

            # ---------- AllGather first (Pool dispatches it, then keeps
            # generating ad2-gather descriptors while it runs) ----------
            nc.gpsimd.collective_compute(
                "AllGather", OP.bypass, ins=[agi2[:]], outs=[ago[:]],
                replica_groups=[list(range(8))])

            # ---------- ad2-gather burst (overlaps the AllGather) ----------
            with tc.tile_pool(name="pad2", bufs=1) as pd2:
                aix2 = pd2.tile([128, NSLOT // 16], i16)
                nc.sync.dma_start(aix2[:], adidx_d[:])
                for ci, (slot0, nids) in enumerate(plan["adcalls"]):
                    _gather_small(
                        nc.gpsimd,
                        at2_all[:, slot0 // 128:(slot0 + nids) // 128, :],
                        agi2[:, 40:42],
                        aix2[:, slot0 // 16:(slot0 + nids) // 16],
                        nids, 2, 128, queue_num=ci % 4)

            edge_pass(2)

    nc.finalize()
    return nc


def kernel(**inputs):
    per_core, plan = _host_prep(**inputs)
    if "nc" not in _CACHE:
        _CACHE["nc"] = _build_nc(plan)
    nc = _CACHE["nc"]
    from concourse.bass_utils import run_bass_kernel_spmd
    res = run_bass_kernel_spmd(nc, per_core, list(range(8)))
    full = np.concatenate([res.results[c]["out"] for c in range(8)], axis=0)
    return np.ascontiguousarray(full[:N]).astype(np.float32)


# revision 43
# speedup vs baseline: 1.0020x; 1.0020x over previous
"""GAT (2-layer PyG GATConv, eval) on 8 Trainium2 NeuronCores.

Sharding: nodes range-partitioned (NLOC=12800/core); core c owns edges whose
dst is in its range. Both layers' node tables are computed SHARDED (each core
transforms only its own 12800-node block) and replicated by one AllGather per
layer; each AllGather is fully overlapped by the a_dst gather burst for the
next edge pass, which reads only the local block.

Slot layout per core: superblock (10 windows) -> quadrant -> window, with
per-(window,quadrant) STATIC capacities = max edge count over the 8 cores
(SPMD: one module runs on all cores; only tensor contents differ) — ~10%
slot padding vs 28% for fixed-size groups. Every dma_gather call carries at
most 1024 indices (the SWDGE ucode's ring limit on real hardware; larger
calls hard-crash the device), rotated across the 4 SWDGE queues.

Per layer, per edge slot: a 144B/82B payload gather pulls [h|a_src] rows
(256B-stride tables, int16 idx into 25600-row quadrants); a 16B/4B gather
pulls a_dst. Softmax is the shift-invariant no-max form
(w = exp(leakyrelu(as+ad)), |e| < ~25 so fp32 exp is safe). Segment reduction
is a PE matmul whose stationary matrix is a transposed one-hot built by DVE
is_equal in the 2x-mode layout (both operands packed 2-byte, last-dim stride
1); chunk slices address it with a strided free dim, boundary chunks carry a
masked drel column per touching window. Messages msg = h*w use an
Act-expanded weight tile so the DVE mult also runs in 2x mode. Softmax
normalize + ELU + the r2 = hlT @ [W2|a2_src|a2_dst] projection are batched
per superblock; tables are written node-permuted (row = p*100 + j within
each core block) so table writes coalesce to one descriptor per partition.
"""
import numpy as np
import ml_dtypes

N = 100000
E = 1600000
NF = 256
HEADS, NHID = 8, 8
NH = HEADS * NHID          # 64
NCLASS = 40
NLOC = 12800               # nodes per core
NW = 100                   # 128-dst windows per core
NQ = 4                     # src table quadrants
QS = 25600                 # rows per quadrant
NWSB = 10                  # windows per superblock
NSB = NW // NWSB           # 10 superblocks
NTOT = 102400
ACC_EPS = 1e-16

_CACHE = {}


def _ceil128(x):
    return (x + 127) & ~127


def _host_prep(x, edge_index, W1, a1_src, a1_dst, b1, W2, a2_src, a2_dst, b2):
    src = np.asarray(edge_index[0], dtype=np.int64)
    dst = np.asarray(edge_index[1], dtype=np.int64)

    # table-row permutation: node n -> row  c*NLOC + (l%128)*NW + l//128
    def rowperm(n):
        c = n // NLOC
        l = n - c * NLOC
        return c * NLOC + (l % 128) * NW + l // 128

    srow = rowperm(src)
    sq = srow // QS
    sidx = (srow - sq * QS).astype(np.int16)

    core = dst // NLOC
    dloc = dst - core * NLOC
    w_e = (dloc >> 7).astype(np.int64)
    dr_e = (dloc & 127).astype(np.int64)
    adidx = (dr_e * NW + w_e).astype(np.int16)
    sb_e = w_e // NWSB

    # static capacities: max over cores per (window, quadrant)
    gkey = (core * NW + w_e) * NQ + sq          # [E]
    cnt = np.bincount(gkey, minlength=8 * NW * NQ).reshape(8, NW, NQ)
    cap = cnt.max(axis=0)                        # [NW, NQ]

    # slot layout: sb -> quadrant -> window.  SWDGE gather ucode dies above
    # 1024 indices per call (empirical), so every call is <= 1024.
    MAXIDX = 1024
    wq_start = np.zeros((NW, NQ), np.int64)
    hcalls = []                                  # (sb, q, slot0, n_idx)
    sb_chunks = []                               # (k0, k1) per sb
    nslot = 0
    for s in range(NSB):
        k0 = nslot // 128
        for q in range(NQ):
            seg0 = nslot
            for w in range(s * NWSB, (s + 1) * NWSB):
                wq_start[w, q] = nslot
                nslot += int(cap[w, q])
            nslot = _ceil128(nslot)
            for off in range(seg0, nslot, MAXIDX):
                hcalls.append((s, q, off, min(MAXIDX, nslot - off)))
        sb_chunks.append((k0, nslot // 128))
    NSLOT = nslot
    NCHUNK = NSLOT // 128

    # dst-indexed gather calls (a_dst expansion), whole slot range
    adcalls = [(off, min(MAXIDX, NSLOT - off))
               for off in range(0, NSLOT, MAXIDX)]

    # per-window chunk columns
    colmap = np.full((NW, NCHUNK), -1, np.int64)
    wcols = []                                   # per w: (colbase, [chunks])
    ncol = 0
    for w in range(NW):
        cols = []
        for q in range(NQ):
            a = int(wq_start[w, q])
            b = a + int(cap[w, q])
            for k in range(a // 128, (b + 127) // 128):
                cols.append(k)
                colmap[w, k] = ncol
                ncol += 1
        wcols.append(cols)
    NCOL = ncol
    MAXCPW = max(len(c) for c in wcols)

    plan = {
        "NSLOT": NSLOT, "NCHUNK": NCHUNK, "NCOL": NCOL, "MAXCPW": MAXCPW,
        "hcalls": hcalls, "adcalls": adcalls, "sb_chunks": sb_chunks,
        "wcols": wcols,
        "skip_b1": bool(np.all(np.asarray(b1) == 0)),
        "skip_b2": bool(np.all(np.asarray(b2) == 0)),
    }

    # group-id in slot order: (sb, q, w_in_sb)
    flatg = (sb_e * NQ + sq) * NWSB + (w_e - sb_e * NWSB)
    gstart_flat = np.zeros(NSB * NQ * NWSB, np.int64)
    for s in range(NSB):
        for q in range(NQ):
            for wi in range(NWSB):
                gstart_flat[(s * NQ + q) * NWSB + wi] = wq_start[s * NWSB + wi, q]

    per_core = []
    hidx_all, adidx_all, drel_all = [], [], []
    for c in range(8):
        m = core == c
        fg = flatg[m]
        drc = dr_e[m]
        order = np.lexsort((drc, fg))
        fgs = fg[order]
        cntc = np.bincount(fgs, minlength=NSB * NQ * NWSB)
        starts = np.zeros_like(cntc)
        starts[1:] = np.cumsum(cntc)[:-1]
        rank = np.arange(len(fgs)) - starts[fgs]
        slot = gstart_flat[fgs] + rank

        hvec = np.zeros(NSLOT, np.int16)
        avec = np.zeros(NSLOT, np.int16)
        hvec[slot] = sidx[m][order]
        avec[slot] = adidx[m][order]

        drel = np.full((128, NCOL), 128.0, np.float32)
        k_s = slot >> 7
        p_s = slot & 127
        we_s = w_e[m][order]
        col_s = colmap[we_s, k_s]
        assert (col_s >= 0).all()
        drel[p_s, col_s] = drc[order].astype(np.float32)

        def wrap16(v):
            o = np.zeros((128, NSLOT // 16), np.int16)
            sl = np.arange(NSLOT)
            o[sl % 16, sl // 16] = v
            for r in range(1, 8):
                o[16 * r:16 * (r + 1)] = o[:16]
            return o

        hidx_all.append(wrap16(hvec))
        adidx_all.append(wrap16(avec))
        drel_all.append(drel.astype(ml_dtypes.bfloat16))

    # weights
    W1 = np.asarray(W1, np.float32)
    v_s1 = np.einsum("chk,hk->ch", W1.reshape(NF, HEADS, NHID),
                     np.asarray(a1_src, np.float32))
    v_d1 = np.einsum("chk,hk->ch", W1.reshape(NF, HEADS, NHID),
                     np.asarray(a1_dst, np.float32))
    W1e = np.concatenate([W1, v_s1, v_d1], axis=1).reshape(2, 128, 80)
    W1e = W1e.astype(np.float16)

    W2 = np.asarray(W2, np.float32)
    v_s2 = W2 @ np.asarray(a2_src, np.float32)[0]
    v_d2 = W2 @ np.asarray(a2_dst, np.float32)[0]
    W2e = np.concatenate([W2, v_s2[:, None], v_d2[:, None]],
                         axis=1).astype(np.float16)   # [64, 42]

    xp = np.zeros((NTOT, NF), np.float32)
    xp[:N] = np.asarray(x, np.float32)

    for c in range(8):
        xloc = np.ascontiguousarray(xp[c * NLOC:(c + 1) * NLOC].T)
        per_core.append({
            "xTloc": xloc.astype(np.float16).reshape(2, 128, NLOC),
            "W1e": W1e,
            "W2e": W2e,
            "b1": np.asarray(b1, np.float32)[None, :],
            "b2": np.asarray(b2, np.float32)[None, :],
            "hidx": hidx_all[c],
            "adidx": adidx_all[c],
            "drel": drel_all[c],
        })
    return per_core, plan


def _gather_small(g, out_ap, in_ap, idxs_ap, num_idxs, elem_size, elem_step,
                  queue_num=0):
    """dma_gather with payload < 256B; only the 256B row-stride rule is real
    for the non-transpose path."""
    import concourse.mybir as mybir
    stride_bytes = elem_step * mybir.dt.size(in_ap.dtype)
    assert stride_bytes % 256 == 0
    _in_ap = g.lower_ap_dma(in_ap, for_custom_bir_dma=True)
    _idxs_ap = g.lower_ap(idxs_ap)
    _out_ap = g.lower_ap(out_ap)
    return g.add_instruction(mybir.InstDMAGatherAnt(
        name=g.bass.get_next_instruction_name(),
        ins=[*_in_ap, _idxs_ap, g.lower_val_access(g.to_reg(num_idxs))],
        outs=[_out_ap],
        transpose=False,
        num_idxs=num_idxs,
        elem_size=elem_size,
        stride_bytes_256=stride_bytes // 256,
        gen_mode=0,
        single_packet=True,
        queue_num=queue_num,
        sbuf_tokens_per_rank=0,
        sbuf_free_dim_per_rank=0,
        sbuf_free_dim_pad_per_rank=0,
        sbuf_byte_offset=0,
    ))


def _build_nc(plan):
    import concourse.bass as bass
    import concourse.bacc as bacc
    import concourse.mybir as mybir
    import concourse.tile as tile
    from concourse.library_config import mlp
    from concourse.masks import make_identity

    f32, f16, bf16, i16 = (mybir.dt.float32, mybir.dt.float16,
                           mybir.dt.bfloat16, mybir.dt.int16)
    AF = mybir.ActivationFunctionType
    OP = mybir.AluOpType

    NSLOT = plan["NSLOT"]
    NCOL = plan["NCOL"]
    MAXCPW = plan["MAXCPW"]
    sb_chunks = plan["sb_chunks"]
    wcols = plan["wcols"]
    CPSB_MAX = max(k1 - k0 for k0, k1 in sb_chunks)
    colbase = [0] * NW
    for w in range(1, NW):
        colbase[w] = colbase[w - 1] + len(wcols[w - 1])

    nc = bacc.Bacc("TRN2", target_bir_lowering=False, debug=False,
                   num_devices=8, num_swdge_queues=4)

    xTloc = nc.dram_tensor("xTloc", [2, 128, NLOC], f16, kind="ExternalInput")
    W1e = nc.dram_tensor("W1e", [2, 128, 80], f16, kind="ExternalInput")
    W2e = nc.dram_tensor("W2e", [64, 42], f16, kind="ExternalInput")
    b1 = nc.dram_tensor("b1", [1, 64], f32, kind="ExternalInput")
    b2 = nc.dram_tensor("b2", [1, 40], f32, kind="ExternalInput")
    hidx_d = nc.dram_tensor("hidx", [128, NSLOT // 16], i16,
                            kind="ExternalInput")
    adidx_d = nc.dram_tensor("adidx", [128, NSLOT // 16], i16,
                             kind="ExternalInput")
    drel_d = nc.dram_tensor("drel", [128, NCOL], bf16, kind="ExternalInput")
    out = nc.dram_tensor("out", [NLOC, 40], f32, kind="ExternalOutput")

    agi1 = nc.dram_tensor("agi1", [NLOC, 128], f16)    # local [h1|as1|ad1|pad]
    tab1 = nc.dram_tensor("tab1", [NTOT, 128], f16, addr_space="Shared")
    agi2 = nc.dram_tensor("agi2", [NLOC, 128], f16)    # [h2|as2|ad2|pad]
    ago = nc.dram_tensor("ago", [NTOT, 128], f16, addr_space="Shared")

    def BC(ap, dims):
        return bass.AP(ap.tensor, ap.offset, dims)

    def dram_rows(t, offset_rows, dims):
        """AP into DRAM tensor t (row-major, 128 f16 cols) at row offset."""
        return bass.AP(t, offset_rows * 128, dims)

    with tile.TileContext(nc) as tc:
        with tc.tile_pool(name="const", bufs=1) as pc:
            nc.gpsimd.load_library(mlp)

            drel_sb = pc.tile([128, NCOL], bf16)
            nc.sync.dma_start(drel_sb[:], drel_d[:])
            w1_sb = pc.tile([128, 2, 80], f16)
            nc.sync.dma_start(w1_sb[:], W1e[:].rearrange("k p n -> p k n"))
            w2_sb = pc.tile([64, 42], f16)
            nc.sync.dma_start(w2_sb[:], W2e[:])

            ii = pc.tile([128, 128, MAXCPW], i16)
            nc.gpsimd.iota(ii[:], pattern=[[1, 128], [0, MAXCPW]], base=0,
                           channel_multiplier=0)
            iota_rep = pc.tile([128, 128, MAXCPW], bf16)
            nc.vector.tensor_copy(out=iota_rep[:], in_=ii[:])

            ident = pc.tile([128, 128], f16)
            make_identity(nc, ident[:])

            ones32 = pc.tile([1, 128], f32)
            nc.vector.memset(ones32[:], 1.0)

            b1r = pc.tile([128, 64], f32)
            b2r = pc.tile([128, 40], f32)
            with tc.tile_pool(name="pini", bufs=2, space="PSUM") as ppi:
                for row_d, width, dest in ((b1, 64, b1r), (b2, 40, b2r)):
                    t = pc.tile([1, width], f32, tag=f"rrow{width}")
                    nc.sync.dma_start(t[:], row_d[:])
                    ps = ppi.tile([128, width], f32, tag="rep")
                    nc.tensor.matmul(ps[:], lhsT=ones32[:], rhs=t[:],
                                     start=True, stop=True)
                    nc.vector.tensor_copy(out=dest[:], in_=ps[:])

            # a_dst per-slot tiles, gathered once and reused per layer
            at1_all = pc.tile([128, plan["NCHUNK"], 8], f16)
            at2_all = pc.tile([128, plan["NCHUNK"], 2], f16)

            # ---------- phase A (sharded): each core transforms only its own
            # node block -> agi1, AllGather -> tab1; the ad1-gather burst runs
            # during the collective (it reads the local agi1) ----------
            with (tc.tile_pool(name="pa", bufs=3) as pa,
                  tc.tile_pool(name="ppa", bufs=2, space="PSUM") as ppa):
                AB = 20                      # chunks per DMA batch
                for jj in range(0, 100, AB):
                    nb = min(AB, 100 - jj)
                    xt = pa.tile([128, 2, AB * 128], f16, tag="xt")
                    for k in range(2):
                        nc.sync.dma_start(
                            xt[:, k, 0:nb * 128],
                            xTloc[k, :, jj * 128:(jj + nb) * 128])
                    row = pa.tile([128, AB, 128], f16, tag="row")
                    for u in range(0, nb, 4):
                        ub = min(4, nb - u)
                        ps = ppa.tile([128, 4, 80], f32, tag="np1")
                        for j in range(ub):
                            for k in range(2):
                                nc.tensor.matmul(
                                    ps[:, j, :],
                                    lhsT=xt[:, k,
                                            (u + j) * 128:(u + j + 1) * 128],
                                    rhs=w1_sb[:, k, :], start=(k == 0),
                                    stop=(k == 1))
                        if (u // 4) % 2:
                            nc.vector.tensor_copy(out=row[:, u:u + ub, 0:80],
                                                  in_=ps[:, 0:ub, :])
                        else:
                            nc.scalar.copy(out=row[:, u:u + ub, 0:80],
                                           in_=ps[:, 0:ub, :])
                    nc.sync.dma_start(
                        dram_rows(agi1, jj,
                                  [[NW * 128, 128], [128, nb], [1, 128]]),
                        row[:, 0:nb, :])

                nc.gpsimd.collective_compute(
                    "AllGather", OP.bypass, ins=[agi1[:]], outs=[tab1[:]],
                    replica_groups=[list(range(8))])

                # ad1-gather burst (reads local agi1; overlaps the AllGather)
                aix = pa.tile([128, NSLOT // 16], i16, tag="aix")
                nc.sync.dma_start(aix[:], adidx_d[:])
                for ci, (slot0, nids) in enumerate(plan["adcalls"]):
                    _gather_small(
                        nc.gpsimd,
                        at1_all[:, slot0 // 128:(slot0 + nids) // 128, :],
                        agi1[:, 72:80],
                        aix[:, slot0 // 16:(slot0 + nids) // 16],
                        nids, 8, 128, queue_num=ci % 4)

            # ---------- phase B: layer-1 edge pass ----------
            def edge_pass(layer):
                if layer == 1:
                    tab, ncols_h, as_col = tab1, 72, 64
                    nheads, msgw = 8, 72
                else:
                    tab, ncols_h, as_col = ago, 41, 40
                    nheads, msgw = 1, 41
                pool_name = f"pe{layer}"
                with (tc.tile_pool(name=pool_name, bufs=2) as pb,
                      tc.tile_pool(name=pool_name + "h", bufs=3) as ph,
                      tc.tile_pool(name=pool_name + "m", bufs=1) as pm,
                      tc.tile_pool(name=pool_name + "w",
                                   bufs=(2 if layer == 1 else 3)) as pw,
                      tc.tile_pool(name=pool_name + "p", bufs=2,
                                   space="PSUM") as ppb):
                    qn = 0
                    for s in range(NSB):
                        k0, k1 = sb_chunks[s]
                        cps = k1 - k0
                        hix = pb.tile([128, CPSB_MAX * 8], i16, tag="hix")
                        nc.sync.dma_start(hix[:, 0:cps * 8],
                                          hidx_d[:, k0 * 8:k1 * 8])

                        ht = ph.tile([128, CPSB_MAX, ncols_h], f16, tag="ht")
                        for (ss, q, slot0, nids) in plan["hcalls"]:
                            if ss != s:
                                continue
                            c0 = slot0 // 128 - k0
                            _gather_small(
                                nc.gpsimd,
                                ht[:, c0:c0 + nids // 128, :],
                                tab[q * QS:(q + 1) * QS, 0:ncols_h],
                                hix[:, (slot0 - k0 * 128) // 16:
                                    (slot0 - k0 * 128 + nids) // 16],
                                nids, ncols_h, 128, queue_num=qn % 4)
                            qn += 1
                        if layer == 1:
                            at_s = at1_all[:, k0:k1, :]
                        else:
                            at_s = at2_all[:, k0:k1, 1:2]

                        e = pm.tile([128, CPSB_MAX, nheads], f32, tag="e")
                        lr = e
                        nc.vector.tensor_tensor(
                            out=e[:, 0:cps, :],
                            in0=ht[:, 0:cps, as_col:as_col + nheads],
                            in1=at_s, op=OP.add)
                        nc.vector.scalar_tensor_tensor(
                            out=lr[:, 0:cps, :], in0=e[:, 0:cps, :],
                            scalar=0.2, in1=e[:, 0:cps, :],
                            op0=OP.mult, op1=OP.max)

                        msg = pm.tile([128, CPSB_MAX, msgw], bf16, tag="msg")
                        # w into msg's trailing cols (compact exp)
                        nc.scalar.activation(
                            out=msg[:, 0:cps, as_col:as_col + nheads],
                            in_=lr[:, 0:cps, :], func=AF.Exp)
                        if layer == 1:
                            # expanded weights for a clean 2x-mode mult
                            half = (CPSB_MAX + 1) // 2
                            wgx = pm.tile([128, half, 8, 8], bf16, tag="wgx")
                            for h0 in (0, half):
                                hn = min(half, cps - h0)
                                if hn <= 0:
                                    continue
                                lrs = lr[:, h0:h0 + hn, :]
                                nc.scalar.activation(
                                    out=wgx[:, 0:hn, :, :],
                                    in_=BC(lrs, [lrs.ap[0], lrs.ap[1],
                                                 lrs.ap[2], [0, 8]]),
                                    func=AF.Exp)
                                m_ = msg[:, h0:h0 + hn, 0:64]
                                h_ = ht[:, h0:h0 + hn, 0:64]
                                nc.vector.tensor_tensor(
                                    out=BC(m_, [m_.ap[0], m_.ap[1],
                                                [8, 8], [1, 8]]),
                                    in0=BC(h_, [h_.ap[0], h_.ap[1],
                                                [8, 8], [1, 8]]),
                                    in1=wgx[:, 0:hn, :, :], op=OP.mult)
                        else:
                            wgx2 = pw.tile([128, CPSB_MAX, 40], bf16,
                                           tag="wgx2")
                            lrs = lr[:, 0:cps, :]
                            nc.scalar.activation(
                                out=wgx2[:, 0:cps, :],
                                in_=BC(lrs, [lrs.ap[0], lrs.ap[1], [0, 40]]),
                                func=AF.Exp)
                            nc.vector.tensor_tensor(
                                out=msg[:, 0:cps, 0:40],
                                in0=ht[:, 0:cps, 0:40],
                                in1=wgx2[:, 0:cps, :], op=OP.mult)

                        # windows: one-hot + aggregation matmuls, PSUM
                        # evicted into a per-sb batch tile
                        hsb = pm.tile([128, NWSB, msgw], f32, tag="hsb")
                        for wi in range(NWSB):
                            w = s * NWSB + wi
                            cols = wcols[w]
                            cpw = len(cols)
                            c0 = colbase[w]
                            ohT = pw.tile([128, 128, MAXCPW], bf16, tag="ohT")
                            dr = drel_sb[:, c0:c0 + cpw]
                            nc.vector.tensor_tensor(
                                out=ohT[:, :, 0:cpw],
                                in0=BC(dr, [dr.ap[0], [0, 128], dr.ap[1]]),
                                in1=iota_rep[:, :, 0:cpw], op=OP.is_equal)
                            ps = ppb.tile([128, msgw], f32, tag="agg")
                            for i, k in enumerate(cols):
                                nc.tensor.matmul(
                                    ps[:], lhsT=ohT[:, :, i],
                                    rhs=msg[:, k - k0, :],
                                    start=(i == 0), stop=(i == cpw - 1))
                            nc.scalar.copy(out=hsb[:, wi, :], in_=ps[:])

                        # per-sb batched softmax-normalize (+ elu/r2 for L1)
                        if layer == 1:
                            den = pw.tile([128, NWSB, 8], f32, tag="den")
                            nc.scalar.activation(out=den[:],
                                                 in_=hsb[:, :, 64:72],
                                                 func=AF.Copy, bias=ACC_EPS)
                            rec = pw.tile([128, NWSB, 8], f32, tag="rec")
                            nc.vector.reciprocal(
                                rec[:].rearrange("p a b -> p (a b)"),
                                den[:].rearrange("p a b -> p (a b)"))
                            o1 = pw.tile([128, NWSB, 64], f32, tag="o1")
                            nu = hsb[:, :, 0:64]
                            r_ = rec[:]
                            nc.vector.tensor_tensor(
                                out=BC(o1[:], [o1[:].ap[0], [64, NWSB],
                                               [8, 8], [1, 8]]),
                                in0=BC(nu, [nu.ap[0], [72, NWSB],
                                            [8, 8], [1, 8]]),
                                in1=BC(r_, [r_.ap[0], [8, NWSB],
                                            [1, 8], [0, 8]]),
                                op=OP.mult)
                            o1v = o1[:].rearrange("p a b -> p (a b)")
                            if not plan["skip_b1"]:
                                b1w = b1r[:]
                                nc.vector.tensor_tensor(
                                    out=o1v,
                                    in0=o1v,
                                    in1=BC(b1w, [b1w.ap[0], [0, NWSB],
                                                 [1, 64]]),
                                    op=OP.add)
                            # elu = relu(x) + exp(-relu(-x)) - 1
                            rneg = pw.tile([128, NWSB, 64], f32, tag="rneg")
                            nc.scalar.activation(
                                out=rneg[:].rearrange("p a b -> p (a b)"),
                                in_=o1v, func=AF.Relu, scale=-1.0)
                            expn = rneg
                            nc.scalar.activation(
                                out=expn[:].rearrange("p a b -> p (a b)"),
                                in_=rneg[:].rearrange("p a b -> p (a b)"),
                                func=AF.Exp, scale=-1.0)
                            pos = pw.tile([128, NWSB, 64], f32, tag="pos")
                            nc.scalar.activation(
                                out=pos[:].rearrange("p a b -> p (a b)"),
                                in_=o1v, func=AF.Relu)
                            hl16 = pw.tile([128, NWSB, 64], f16, tag="hl16")
                            nc.vector.scalar_tensor_tensor(
                                out=hl16[:].rearrange("p a b -> p (a b)"),
                                in0=expn[:].rearrange("p a b -> p (a b)"),
                                scalar=-1.0,
                                in1=pos[:].rearrange("p a b -> p (a b)"),
                                op0=OP.add, op1=OP.add)
                            r2s = pw.tile([128, NWSB, 42], f16, tag="r2s")
                            for wi in range(NWSB):
                                pst = ppb.tile([64, 128], f16, tag="tr")
                                nc.tensor.transpose(out=pst[:],
                                                    in_=hl16[:, wi, :],
                                                    identity=ident[:])
                                hlT = pw.tile([64, 128], f16, tag="hlT")
                                nc.scalar.copy(out=hlT[:], in_=pst[:])
                                r2p = ppb.tile([128, 42], f32, tag="r2p")
                                nc.tensor.matmul(r2p[:], lhsT=hlT[:],
                                                 rhs=w2_sb[:], start=True,
                                                 stop=True)
                                nc.scalar.copy(out=r2s[:, wi, :], in_=r2p[:])
                            nc.sync.dma_start(
                                bass.AP(agi2, (s * NWSB) * 128,
                                        [[NW * 128, 128], [128, NWSB],
                                         [1, 42]]),
                                r2s[:])
                        else:
                            den = pw.tile([128, NWSB, 1], f32, tag="den2")
                            nc.scalar.activation(out=den[:],
                                                 in_=hsb[:, :, 40:41],
                                                 func=AF.Copy, bias=ACC_EPS)
                            rec = pw.tile([128, NWSB, 1], f32, tag="rec2")
                            nc.vector.reciprocal(
                                rec[:].rearrange("p a b -> p (a b)"),
                                den[:].rearrange("p a b -> p (a b)"))
                            o2 = pw.tile([128, NWSB, 40], f32, tag="o2")
                            nu = hsb[:, :, 0:40]
                            r_ = rec[:]
                            nc.vector.tensor_tensor(
                                out=o2[:],
                                in0=BC(nu, [nu.ap[0], [41, NWSB], [1, 40]]),
                                in1=BC(r_, [r_.ap[0], [1, NWSB], [0, 40]]),
                                op=OP.mult)
                            o2v = o2[:].rearrange("p a b -> p (a b)")
                            if not plan["skip_b2"]:
                                b2w = b2r[:]
                                nc.vector.tensor_tensor(
                                    out=o2v, in0=o2v,
                                    in1=BC(b2w, [b2w.ap[0], [0, NWSB],
                                                 [1, 40]]),
                                    op=OP.add)
                            nc.sync.dma_start(
                                bass.AP(out, (s * NWSB) * 128 * 40,
                                        [[40, 128], [128 * 40, NWSB],
                                         [1, 40]]),
                                o2[:])

            edge_pass(1)

            # ---------- AllGather first (Pool dispatches it, then keeps
            # generating ad2-gather descriptors while it runs) ----------
            nc.gpsimd.collective_compute(
                "AllGather", OP.bypass, ins=[agi2[:]], outs=[ago[:]],
                replica_groups=[list(range(8))])

            # ---------- ad2-gather burst (overlaps the AllGather) ----------
            with tc.tile_pool(name="pad2", bufs=1) as pd2:
                aix2 = pd2.tile([128, NSLOT // 16], i16)
                nc.sync.dma_start(aix2[:], adidx_d[:])
                for ci, (slot0, nids) in enumerate(plan["adcalls"]):
                    _gather_small(
                        nc.gpsimd,
                        at2_all[:, slot0 // 128:(slot0 + nids) // 128, :],
                        agi2[:, 40:42],
                        aix2[:, slot0 // 16:(slot0 + nids) // 16],
                        nids, 2, 128, queue_num=ci % 4)

            edge_pass(2)

    nc.finalize()
    return nc


def kernel(**inputs):
    per_core, plan = _host_prep(**inputs)
    if "nc" not in _CACHE:
        _CACHE["nc"] = _build_nc(plan)
    nc = _CACHE["nc"]
    from concourse.bass_utils import run_bass_kernel_spmd
    res = run_bass_kernel_spmd(nc, per_core, list(range(8)))
    full = np.concatenate([res.results[c]["out"] for c in range(8)], axis=0)
    return np.ascontiguousarray(full[:N]).astype(np.float32)
